# revision 12
# baseline (speedup 1.0000x reference)
"""CTSelectiveSSMBlock Trainium2 kernel.

Strategy: data-parallel over batch (B=8 -> one batch element per NeuronCore).
Per core pipeline over t-tiles of 512 tokens:
  - DVE casts x / delta_t tiles to bf16 (delta_t fused with the max(dt,eps)
    clamp), bounced through DRAM and DMA-transposed into [d, t] layout
    (keeps the tensor engine free of transpose work)
  - GEMM1/2 (bf16): update/gate pre-activations in [e, t] layout
  - ACT: lambda = exp(-decay*dt_clamped) (batched), sigmoid for the gate
  - DVE scalar_tensor_tensor: tmp = gate*(update+b_in); drv = (lam-1)*tmp
  - hardware tensor_tensor_scan for the recurrence (per 128-channel chunk,
    chained across tiles via the previous tile's last column)
  - GEMM3 (bf16): y = states @ -(I + W_out^T) using the negated-drive trick,
    bias b_out added via a K=1 ones-row matmul
  - LayerNorm via bn_stats/bn_aggr + ACT normalize, natural-layout DMA out
"""

import os
import sys
import types
import numpy as np
from contextlib import ExitStack

for _p in ("/opt/trn_rl_repo", "/opt/pypackages"):
    if _p not in sys.path and os.path.isdir(_p):
        sys.path.append(_p)

import concourse.bass as bass  # noqa: E402
import concourse.tile as tile  # noqa: E402
from concourse import bacc, mybir  # noqa: E402

F32 = mybir.dt.float32
BF16 = mybir.dt.float16  # 2-byte compute dtype (fp16: better mantissa than bf16, range is ample here)
AF = mybir.ActivationFunctionType
OP = mybir.AluOpType

B, T, D = 8, 4096, 1024
TT = 512           # tokens per tile
NA = TT // 128     # 128-token sub-chunks per tile
KC = D // 128      # channel chunks
EPS_DT = 1e-4
LN_EPS = 1e-5

# set by test harness for profiling; grading path leaves these off
TRACE = False
LAST_RESULTS = None

_BUILD_CACHE = {}


def _build(nt: int, apply_gb: bool):
    Tl = nt * TT
    nc = bacc.Bacc("TRN2", target_bir_lowering=False, debug=False)

    x_d = nc.dram_tensor("x", [Tl, D], F32, kind="ExternalInput")
    dt_d = nc.dram_tensor("dt", [Tl, D], F32, kind="ExternalInput")
    wig_d = nc.dram_tensor("w_ig", [D, 2 * D], BF16, kind="ExternalInput")
    wom_d = nc.dram_tensor("w_om", [D, D], BF16, kind="ExternalInput")
    big_d = nc.dram_tensor("b_ig", [128, 2 * KC], F32, kind="ExternalInput")
    nd_d = nc.dram_tensor("ndec", [128, KC], F32, kind="ExternalInput")
    bout_d = nc.dram_tensor("b_out", [1, D], BF16, kind="ExternalInput")
    if apply_gb:
        gam_d = nc.dram_tensor("gam", [128, D], F32, kind="ExternalInput")
        bet_d = nc.dram_tensor("bet", [128, D], F32, kind="ExternalInput")
    out_d = nc.dram_tensor("out", [Tl, D], F32, kind="ExternalOutput")
    fs_d = nc.dram_tensor("fs", [128, KC], BF16, kind="ExternalOutput")

    with ExitStack() as ctx:
        tc = ctx.enter_context(tile.TileContext(nc))
        wp = ctx.enter_context(tc.tile_pool(name="wp", bufs=1))
        xin = ctx.enter_context(tc.tile_pool(name="xin", bufs=4))
        xbp = ctx.enter_context(tc.tile_pool(name="xbp", bufs=4))
        drp = ctx.enter_context(tc.tile_pool(name="drp", bufs=2, space="DRAM"))
        xtp = ctx.enter_context(tc.tile_pool(name="xtp", bufs=2))
        stp = ctx.enter_context(tc.tile_pool(name="stp", bufs=2))
        lmp = ctx.enter_context(tc.tile_pool(name="lmp", bufs=10))
        ck3 = ctx.enter_context(tc.tile_pool(name="ck3", bufs=3))
        onp = ctx.enter_context(tc.tile_pool(name="onp", bufs=3))
        sm = ctx.enter_context(tc.tile_pool(name="sm", bufs=4))
        ug_ps = ctx.enter_context(tc.tile_pool(name="ugps", bufs=4, space="PSUM"))
        y_ps = ctx.enter_context(tc.tile_pool(name="yps", bufs=4, space="PSUM"))

        # ---- persistent weights / constants ----
        wig_s = wp.tile([128, KC * 2 * D], BF16, tag="wig")
        nc.sync.dma_start(
            wig_s[:].rearrange("p (k e) -> p k e", k=KC),
            wig_d.rearrange("(k p) e -> p k e", p=128),
        )
        wom_s = wp.tile([128, KC * D], BF16, tag="wom")
        nc.sync.dma_start(
            wom_s[:].rearrange("p (k e) -> p k e", k=KC),
            wom_d.rearrange("(k p) e -> p k e", p=128),
        )
        big_s = wp.tile([128, 2 * KC], F32, tag="big")
        nc.sync.dma_start(big_s[:], big_d[:])
        nd_s = wp.tile([128, KC], F32, tag="nd")
        nc.sync.dma_start(nd_s[:], nd_d[:])
        bout_s = wp.tile([1, D], BF16, tag="bout")
        nc.sync.dma_start(bout_s[:], bout_d[:])
        ones_s = wp.tile([1, 128], BF16, tag="ones")
        nc.vector.memset(ones_s[:], 1.0)
        if apply_gb:
            gam_s = wp.tile([128, D], F32, tag="gam")
            nc.sync.dma_start(gam_s[:], gam_d[:])
            bet_s = wp.tile([128, D], F32, tag="bet")
            nc.sync.dma_start(bet_s[:], bet_d[:])

        sT_prev = None
        for it in range(nt):
            t0 = it * TT
            # stage x/dt to bf16 in DRAM (a-chunk granular), then DMA-transpose
            xb_t = drp.tile([TT, D], BF16, tag="xb")
            db_t = drp.tile([TT, D], BF16, tag="db")
            for a in range(NA):
                x_nat = xin.tile([128, D], F32, tag="x_nat")
                nc.sync.dma_start(x_nat[:], x_d[t0 + a * 128: t0 + (a + 1) * 128, :])
                x_b = xbp.tile([128, D], BF16, tag="x_b")
                nc.vector.tensor_copy(x_b[:], x_nat[:])
                nc.sync.dma_start(xb_t[a * 128:(a + 1) * 128, :], x_b[:])
                dt_nat = xin.tile([128, D], F32, tag="dt_nat")
                nc.sync.dma_start(dt_nat[:], dt_d[t0 + a * 128: t0 + (a + 1) * 128, :])
                dt_b = xbp.tile([128, D], BF16, tag="dt_b")
                nc.vector.tensor_scalar_max(dt_b[:], dt_nat[:], EPS_DT)
                nc.sync.dma_start(db_t[a * 128:(a + 1) * 128, :], dt_b[:])

            xt = xtp.tile([128, KC * TT], BF16, tag="xt")
            dtT = xtp.tile([128, KC * TT], BF16, tag="dtT")
            lams = []
            for k in range(KC):
                nc.sync.dma_start(
                    xt[:, k * TT:(k + 1) * TT],
                    xb_t[:, k * 128:(k + 1) * 128], transpose=True,
                )
                nc.sync.dma_start(
                    dtT[:, k * TT:(k + 1) * TT],
                    db_t[:, k * 128:(k + 1) * 128], transpose=True,
                )
                lam = lmp.tile([128, TT], BF16, tag="lam")
                nc.scalar.activation(
                    lam[:], dtT[:, k * TT:(k + 1) * TT], AF.Exp,
                    scale=nd_s[:, k:k + 1],
                )
                lams.append(lam)

            sT = stp.tile([128, KC * TT], BF16, tag="sT")
            for c in range(KC):
                # update pre-activation (chunk c of output channels)
                psu = ug_ps.tile([128, TT], F32, tag="ug")
                for k in range(KC):
                    nc.tensor.matmul(
                        psu[:],
                        wig_s[:, k * 2 * D + c * 128: k * 2 * D + (c + 1) * 128],
                        xt[:, k * TT:(k + 1) * TT],
                        start=(k == 0), stop=(k == KC - 1),
                    )
                # gate pre-activation
                psg = ug_ps.tile([128, TT], F32, tag="ug")
                for k in range(KC):
                    nc.tensor.matmul(
                        psg[:],
                        wig_s[:, k * 2 * D + D + c * 128: k * 2 * D + D + (c + 1) * 128],
                        xt[:, k * TT:(k + 1) * TT],
                        start=(k == 0), stop=(k == KC - 1),
                    )
                gat = ck3.tile([128, TT], BF16, tag="gat")
                nc.scalar.activation(gat[:], psg[:], AF.Sigmoid,
                                     bias=big_s[:, KC + c:KC + c + 1])
                # tmp = gate * (update_pre + b_in)   (reads PSUM directly)
                tmp = ck3.tile([128, TT], BF16, tag="tmp")
                nc.vector.scalar_tensor_tensor(
                    tmp[:], psu[:], big_s[:, c:c + 1], gat[:], op0=OP.add, op1=OP.mult
                )
                # drv = (lam - 1) * tmp  == -drive
                drv = ck3.tile([128, TT], BF16, tag="drv")
                nc.vector.scalar_tensor_tensor(
                    drv[:], lams[c][:], 1.0, tmp[:], op0=OP.subtract, op1=OP.mult
                )
                init = 0.0 if it == 0 else sT_prev[:, c * TT + TT - 1: c * TT + TT]
                nc.vector.tensor_tensor_scan(
                    sT[:, c * TT:(c + 1) * TT], lams[c][:], drv[:], init,
                    op0=OP.mult, op1=OP.add,
                )

            for a in range(NA):
                on = onp.tile([128, D], F32, tag="on")
                yps = []
                for h in range(2):
                    yp = y_ps.tile([128, 512], F32, tag="y")
                    for k in range(KC):
                        nc.tensor.matmul(
                            yp[:],
                            sT[:, k * TT + a * 128: k * TT + (a + 1) * 128],
                            wom_s[:, k * D + h * 512: k * D + (h + 1) * 512],
                            start=(k == 0), stop=False,
                        )
                    nc.tensor.matmul(
                        yp[:], ones_s[:], bout_s[:, h * 512:(h + 1) * 512],
                        start=False, stop=True,
                    )
                    yps.append(yp)
                stats = sm.tile([128, 12], F32, tag="stats")
                nc.vector.bn_stats(stats[:, 0:6], yps[0][:])
                nc.vector.bn_stats(stats[:, 6:12], yps[1][:])
                mv = sm.tile([128, 2], F32, tag="mv")
                nc.vector.bn_aggr(mv[:], stats[:])
                ve = sm.tile([128, 1], F32, tag="ve")
                nc.vector.tensor_scalar_add(ve[:], mv[:, 1:2], LN_EPS)
                sd = sm.tile([128, 1], F32, tag="sd")
                nc.scalar.sqrt(sd[:], ve[:])
                rs = sm.tile([128, 1], F32, tag="rs")
                nc.vector.reciprocal(rs[:], sd[:])
                nm = sm.tile([128, 1], F32, tag="nm")
                nc.vector.scalar_tensor_tensor(
                    nm[:], mv[:, 0:1], -1.0, rs[:], op0=OP.mult, op1=OP.mult
                )
                for h in range(2):
                    nc.scalar.activation(
                        on[:, h * 512:(h + 1) * 512], yps[h][:], AF.Identity,
                        scale=rs[:], bias=nm[:],
                    )
                if apply_gb:
                    nc.vector.tensor_mul(on[:], on[:], gam_s[:])
                    nc.vector.tensor_add(on[:], on[:], bet_s[:])
                nc.sync.dma_start(out_d[t0 + a * 128: t0 + (a + 1) * 128, :], on[:])

            sT_prev = sT

        nc.sync.dma_start(
            fs_d.rearrange("p (k o) -> p k o", o=1),
            sT_prev[:].rearrange("p (k t) -> p k t", k=KC)[:, :, TT - 1:TT],
        )

    nc.compile()
    return nc


def get_nc(nt: int = T // TT, apply_gb: bool = False):
    key = (nt, apply_gb)
    if key not in _BUILD_CACHE:
        _BUILD_CACHE[key] = _build(nt, apply_gb)
    return _BUILD_CACHE[key]


def host_prep(log_decay, W_in, b_in, W_gate, b_gate, W_out, b_out, gamma, beta):
    f32 = np.float32
    bf16 = np.dtype(mybir.dt.np(BF16))
    decay = np.logaddexp(log_decay.astype(f32), f32(0.0)).astype(f32) + f32(1e-4)
    ndec = np.ascontiguousarray((-decay).reshape(KC, 128).T)
    w_ig = np.ascontiguousarray(
        np.concatenate([np.asarray(W_in, f32).T, np.asarray(W_gate, f32).T], axis=1)
    ).astype(bf16)
    w_om = np.ascontiguousarray(
        -(np.eye(D, dtype=f32) + np.asarray(W_out, f32).T)
    ).astype(bf16)
    b_ig = np.ascontiguousarray(
        np.concatenate(
            [np.asarray(b_in, f32).reshape(KC, 128).T,
             np.asarray(b_gate, f32).reshape(KC, 128).T], axis=1)
    )
    b_out_row = np.ascontiguousarray(np.asarray(b_out, f32).reshape(1, D)).astype(bf16)
    gamma = np.asarray(gamma, f32)
    beta = np.asarray(beta, f32)
    apply_gb = not (np.all(gamma == 1.0) and np.all(beta == 0.0))
    common = {
        "w_ig": w_ig, "w_om": w_om, "b_ig": b_ig, "ndec": ndec,
        "b_out": b_out_row,
    }
    if apply_gb:
        common["gam"] = np.ascontiguousarray(np.broadcast_to(gamma, (128, D)))
        common["bet"] = np.ascontiguousarray(np.broadcast_to(beta, (128, D)))
    return common, apply_gb


def kernel(x, delta_t, log_decay, W_in, b_in, W_gate, b_gate, W_out, b_out,
           gamma, beta):
    global LAST_RESULTS
    x = np.ascontiguousarray(np.asarray(x, np.float32))
    delta_t = np.ascontiguousarray(np.asarray(delta_t, np.float32))
    common, apply_gb = host_prep(log_decay, W_in, b_in, W_gate, b_gate,
                                 W_out, b_out, gamma, beta)
    nc = get_nc(T // TT, apply_gb)

    in_maps = []
    for b in range(B):
        m = dict(common)
        m["x"] = x[b]
        m["dt"] = delta_t[b]
        in_maps.append(m)

    kwargs = {}
    if TRACE:
        _install_trace_shims()
        kwargs["trace"] = True

    from concourse.bass_utils import run_bass_kernel_spmd
    res = run_bass_kernel_spmd(nc, in_maps, core_ids=list(range(B)), **kwargs)
    LAST_RESULTS = res

    out = np.stack([res.results[b]["out"] for b in range(B)]).astype(np.float32)
    fs = np.stack(
        [-res.results[b]["fs"].astype(np.float32).T.reshape(D) for b in range(B)]
    ).astype(np.float32)
    return out, fs


def _install_trace_shims():
    """Register the NTFF profile hook (missing antenv.axon_hooks shim) and
    skip the artifact upload. Only used when TRACE is enabled by test.py."""
    try:
        import antenv.axon_hooks  # noqa: F401
        return
    except ImportError:
        pass
    from trn_agent_boot.trn_boot import _ntff_profile_via_ctypes
    hook = _ntff_profile_via_ctypes("/opt/axon/libaxon_pjrt.so")
    mod = types.ModuleType("antenv.axon_hooks")
    mod.get_axon_ntff_profile_hook = lambda: hook
    sys.modules["antenv.axon_hooks"] = mod
    import concourse.bass_utils as bu
    bu.upload_artifacts = lambda tmpdir: "(upload skipped)"


# revision 13
# speedup vs baseline: 1.0852x; 1.0852x over previous
"""CTSelectiveSSMBlock Trainium2 kernel.

Strategy: data-parallel over batch (B=8 -> one batch element per NeuronCore).
Per core pipeline over t-tiles of 512 tokens:
  - DVE casts x / delta_t tiles to fp16 (delta_t fused with the max(dt,eps)
    clamp); PE transposes the fp16 tiles into [d, t] layout (1 cycle/row)
  - GEMM1/2 (fp16): update/gate pre-activations in [e, t] layout
  - ACT: lambda = exp(-decay*dt) straight from the transpose PSUM bank,
    sigmoid for the gate
  - DVE scalar_tensor_tensor: tmp = gate*(update+b_in); drv = (lam-1)*tmp
  - hardware tensor_tensor_scan for the recurrence (per 128-channel chunk,
    chained across tiles via the previous tile's last column)
  - GEMM3 (fp16): y = states @ -(I + W_out^T) using the negated-drive trick,
    bias b_out added via a K=1 ones-row matmul
  - LayerNorm via bn_stats/bn_aggr + ACT normalize, natural-layout DMA out
fp16 (not bf16): same PE/DMA cost, 4 extra mantissa bits; all tensors here
are O(1) so the reduced exponent range is irrelevant.
"""

import os
import sys
import types
import numpy as np
from contextlib import ExitStack

for _p in ("/opt/trn_rl_repo", "/opt/pypackages"):
    if _p not in sys.path and os.path.isdir(_p):
        sys.path.append(_p)

import concourse.bass as bass  # noqa: E402
import concourse.tile as tile  # noqa: E402
from concourse import bacc, mybir  # noqa: E402

F32 = mybir.dt.float32
F16 = mybir.dt.float16
AF = mybir.ActivationFunctionType
OP = mybir.AluOpType

B, T, D = 8, 4096, 1024
TT = 512           # tokens per tile
NA = TT // 128     # 128-token sub-chunks per tile
KC = D // 128      # channel chunks
EPS_DT = 1e-4
LN_EPS = 1e-5

# set by test harness for profiling; grading path leaves these off
TRACE = False
LAST_RESULTS = None

_BUILD_CACHE = {}


def _build(nt: int, apply_gb: bool):
    Tl = nt * TT
    nc = bacc.Bacc("TRN2", target_bir_lowering=False, debug=False)

    x_d = nc.dram_tensor("x", [Tl, D], F32, kind="ExternalInput")
    dt_d = nc.dram_tensor("dt", [Tl, D], F32, kind="ExternalInput")
    wig_d = nc.dram_tensor("w_ig", [D, 2 * D], F16, kind="ExternalInput")
    wom_d = nc.dram_tensor("w_om", [D, D], F16, kind="ExternalInput")
    big_d = nc.dram_tensor("b_ig", [128, 2 * KC], F32, kind="ExternalInput")
    nd_d = nc.dram_tensor("ndec", [128, KC], F32, kind="ExternalInput")
    bout_d = nc.dram_tensor("b_out", [1, D], F16, kind="ExternalInput")
    id_d = nc.dram_tensor("ident", [128, 128], F16, kind="ExternalInput")
    if apply_gb:
        gam_d = nc.dram_tensor("gam", [128, D], F32, kind="ExternalInput")
        bet_d = nc.dram_tensor("bet", [128, D], F32, kind="ExternalInput")
    out_d = nc.dram_tensor("out", [Tl, D], F32, kind="ExternalOutput")
    fs_d = nc.dram_tensor("fs", [128, KC], F16, kind="ExternalOutput")

    with ExitStack() as ctx:
        tc = ctx.enter_context(tile.TileContext(nc))
        wp = ctx.enter_context(tc.tile_pool(name="wp", bufs=1))
        xin = ctx.enter_context(tc.tile_pool(name="xin", bufs=4))
        xbp = ctx.enter_context(tc.tile_pool(name="xbp", bufs=2))
        xtp = ctx.enter_context(tc.tile_pool(name="xtp", bufs=2))
        stp = ctx.enter_context(tc.tile_pool(name="stp", bufs=2))
        lmp = ctx.enter_context(tc.tile_pool(name="lmp", bufs=10))
        ck3 = ctx.enter_context(tc.tile_pool(name="ck3", bufs=3))
        onp = ctx.enter_context(tc.tile_pool(name="onp", bufs=3))
        sm = ctx.enter_context(tc.tile_pool(name="sm", bufs=4))
        pt_ps = ctx.enter_context(tc.tile_pool(name="ptps", bufs=3, space="PSUM"))
        ug_ps = ctx.enter_context(tc.tile_pool(name="ugps", bufs=2, space="PSUM"))
        y_ps = ctx.enter_context(tc.tile_pool(name="yps", bufs=3, space="PSUM"))

        # ---- persistent weights / constants ----
        wig_s = wp.tile([128, KC * 2 * D], F16, tag="wig")
        nc.sync.dma_start(
            wig_s[:].rearrange("p (k e) -> p k e", k=KC),
            wig_d.rearrange("(k p) e -> p k e", p=128),
        )
        wom_s = wp.tile([128, KC * D], F16, tag="wom")
        nc.sync.dma_start(
            wom_s[:].rearrange("p (k e) -> p k e", k=KC),
            wom_d.rearrange("(k p) e -> p k e", p=128),
        )
        big_s = wp.tile([128, 2 * KC], F32, tag="big")
        nc.sync.dma_start(big_s[:], big_d[:])
        nd_s = wp.tile([128, KC], F32, tag="nd")
        nc.sync.dma_start(nd_s[:], nd_d[:])
        bout_s = wp.tile([1, D], F16, tag="bout")
        nc.sync.dma_start(bout_s[:], bout_d[:])
        id_s = wp.tile([128, 128], F16, tag="id")
        nc.sync.dma_start(id_s[:], id_d[:])
        ones_s = wp.tile([1, 128], F16, tag="ones")
        nc.vector.memset(ones_s[:], 1.0)
        if apply_gb:
            gam_s = wp.tile([128, D], F32, tag="gam")
            nc.sync.dma_start(gam_s[:], gam_d[:])
            bet_s = wp.tile([128, D], F32, tag="bet")
            nc.sync.dma_start(bet_s[:], bet_d[:])

        sT_prev = None
        for it in range(nt):
            t0 = it * TT
            # load + cast to fp16 (dt fused with the EPS clamp)
            x_b = xbp.tile([128, NA * D], F16, tag="x_b")
            dt_b = xbp.tile([128, NA * D], F16, tag="dt_b")
            for a in range(NA):
                x_nat = xin.tile([128, D], F32, tag="x_nat")
                nc.sync.dma_start(x_nat[:], x_d[t0 + a * 128: t0 + (a + 1) * 128, :])
                nc.vector.tensor_copy(x_b[:, a * D:(a + 1) * D], x_nat[:])
                dt_nat = xin.tile([128, D], F32, tag="dt_nat")
                nc.sync.dma_start(dt_nat[:], dt_d[t0 + a * 128: t0 + (a + 1) * 128, :])
                nc.vector.tensor_scalar_max(dt_b[:, a * D:(a + 1) * D], dt_nat[:], EPS_DT)

            xt = xtp.tile([128, KC * TT], F16, tag="xt")
            lams = []
            for k in range(KC):
                # x^T chunk: transpose NA fp16 blocks into one PSUM tile
                ptx = pt_ps.tile([128, TT], F16, tag="pt")
                for a in range(NA):
                    nc.tensor.transpose(
                        ptx[:, a * 128:(a + 1) * 128],
                        x_b[:, a * D + k * 128: a * D + (k + 1) * 128],
                        id_s[:],
                    )
                nc.vector.tensor_copy(xt[:, k * TT:(k + 1) * TT], ptx[:])
                # dt^T chunk: lambda computed by ACT straight from PSUM
                ptd = pt_ps.tile([128, TT], F16, tag="pt")
                for a in range(NA):
                    nc.tensor.transpose(
                        ptd[:, a * 128:(a + 1) * 128],
                        dt_b[:, a * D + k * 128: a * D + (k + 1) * 128],
                        id_s[:],
                    )
                lam = lmp.tile([128, TT], F16, tag="lam")
                nc.scalar.activation(lam[:], ptd[:], AF.Exp, scale=nd_s[:, k:k + 1])
                lams.append(lam)

            sT = stp.tile([128, KC * TT], F16, tag="sT")
            for c in range(KC):
                # update pre-activation (chunk c of output channels)
                psu = ug_ps.tile([128, TT], F32, tag="ug")
                for k in range(KC):
                    nc.tensor.matmul(
                        psu[:],
                        wig_s[:, k * 2 * D + c * 128: k * 2 * D + (c + 1) * 128],
                        xt[:, k * TT:(k + 1) * TT],
                        start=(k == 0), stop=(k == KC - 1),
                    )
                # gate pre-activation
                psg = ug_ps.tile([128, TT], F32, tag="ug")
                for k in range(KC):
                    nc.tensor.matmul(
                        psg[:],
                        wig_s[:, k * 2 * D + D + c * 128: k * 2 * D + D + (c + 1) * 128],
                        xt[:, k * TT:(k + 1) * TT],
                        start=(k == 0), stop=(k == KC - 1),
                    )
                gat = ck3.tile([128, TT], F16, tag="gat")
                nc.scalar.activation(gat[:], psg[:], AF.Sigmoid,
                                     bias=big_s[:, KC + c:KC + c + 1])
                # tmp = gate * (update_pre + b_in)   (reads PSUM directly)
                tmp = ck3.tile([128, TT], F16, tag="tmp")
                nc.vector.scalar_tensor_tensor(
                    tmp[:], psu[:], big_s[:, c:c + 1], gat[:], op0=OP.add, op1=OP.mult
                )
                # drv = (lam - 1) * tmp  == -drive
                drv = ck3.tile([128, TT], F16, tag="drv")
                nc.vector.scalar_tensor_tensor(
                    drv[:], lams[c][:], 1.0, tmp[:], op0=OP.subtract, op1=OP.mult
                )
                init = 0.0 if it == 0 else sT_prev[:, c * TT + TT - 1: c * TT + TT]
                nc.vector.tensor_tensor_scan(
                    sT[:, c * TT:(c + 1) * TT], lams[c][:], drv[:], init,
                    op0=OP.mult, op1=OP.add,
                )

            for a in range(NA):
                on = onp.tile([128, D], F32, tag="on")
                yps = []
                for h in range(2):
                    yp = y_ps.tile([128, 512], F32, tag="y")
                    for k in range(KC):
                        nc.tensor.matmul(
                            yp[:],
                            sT[:, k * TT + a * 128: k * TT + (a + 1) * 128],
                            wom_s[:, k * D + h * 512: k * D + (h + 1) * 512],
                            start=(k == 0), stop=False,
                        )
                    nc.tensor.matmul(
                        yp[:], ones_s[:], bout_s[:, h * 512:(h + 1) * 512],
                        start=False, stop=True,
                    )
                    yps.append(yp)
                stats = sm.tile([128, 12], F32, tag="stats")
                nc.vector.bn_stats(stats[:, 0:6], yps[0][:])
                nc.vector.bn_stats(stats[:, 6:12], yps[1][:])
                mv = sm.tile([128, 2], F32, tag="mv")
                nc.vector.bn_aggr(mv[:], stats[:])
                ve = sm.tile([128, 1], F32, tag="ve")
                nc.vector.tensor_scalar_add(ve[:], mv[:, 1:2], LN_EPS)
                sd = sm.tile([128, 1], F32, tag="sd")
                nc.scalar.sqrt(sd[:], ve[:])
                rs = sm.tile([128, 1], F32, tag="rs")
                nc.vector.reciprocal(rs[:], sd[:])
                nm = sm.tile([128, 1], F32, tag="nm")
                nc.vector.scalar_tensor_tensor(
                    nm[:], mv[:, 0:1], -1.0, rs[:], op0=OP.mult, op1=OP.mult
                )
                for h in range(2):
                    nc.scalar.activation(
                        on[:, h * 512:(h + 1) * 512], yps[h][:], AF.Identity,
                        scale=rs[:], bias=nm[:],
                    )
                if apply_gb:
                    nc.vector.tensor_mul(on[:], on[:], gam_s[:])
                    nc.vector.tensor_add(on[:], on[:], bet_s[:])
                nc.sync.dma_start(out_d[t0 + a * 128: t0 + (a + 1) * 128, :], on[:])

            sT_prev = sT

        nc.sync.dma_start(
            fs_d.rearrange("p (k o) -> p k o", o=1),
            sT_prev[:].rearrange("p (k t) -> p k t", k=KC)[:, :, TT - 1:TT],
        )

    nc.compile()
    return nc


def get_nc(nt: int = T // TT, apply_gb: bool = False):
    key = (nt, apply_gb)
    if key not in _BUILD_CACHE:
        _BUILD_CACHE[key] = _build(nt, apply_gb)
    return _BUILD_CACHE[key]


def host_prep(log_decay, W_in, b_in, W_gate, b_gate, W_out, b_out, gamma, beta):
    f32 = np.float32
    f16 = np.float16
    decay = np.logaddexp(log_decay.astype(f32), f32(0.0)).astype(f32) + f32(1e-4)
    ndec = np.ascontiguousarray((-decay).reshape(KC, 128).T)
    w_ig = np.ascontiguousarray(
        np.concatenate([np.asarray(W_in, f32).T, np.asarray(W_gate, f32).T], axis=1)
    ).astype(f16)
    w_om = np.ascontiguousarray(
        -(np.eye(D, dtype=f32) + np.asarray(W_out, f32).T)
    ).astype(f16)
    b_ig = np.ascontiguousarray(
        np.concatenate(
            [np.asarray(b_in, f32).reshape(KC, 128).T,
             np.asarray(b_gate, f32).reshape(KC, 128).T], axis=1)
    )
    b_out_row = np.ascontiguousarray(np.asarray(b_out, f32).reshape(1, D)).astype(f16)
    ident = np.eye(128, dtype=f16)
    gamma = np.asarray(gamma, f32)
    beta = np.asarray(beta, f32)
    apply_gb = not (np.all(gamma == 1.0) and np.all(beta == 0.0))
    common = {
        "w_ig": w_ig, "w_om": w_om, "b_ig": b_ig, "ndec": ndec,
        "b_out": b_out_row, "ident": ident,
    }
    if apply_gb:
        common["gam"] = np.ascontiguousarray(np.broadcast_to(gamma, (128, D)))
        common["bet"] = np.ascontiguousarray(np.broadcast_to(beta, (128, D)))
    return common, apply_gb


def kernel(x, delta_t, log_decay, W_in, b_in, W_gate, b_gate, W_out, b_out,
           gamma, beta):
    global LAST_RESULTS
    x = np.ascontiguousarray(np.asarray(x, np.float32))
    delta_t = np.ascontiguousarray(np.asarray(delta_t, np.float32))
    common, apply_gb = host_prep(log_decay, W_in, b_in, W_gate, b_gate,
                                 W_out, b_out, gamma, beta)
    nc = get_nc(T // TT, apply_gb)

    in_maps = []
    for b in range(B):
        m = dict(common)
        m["x"] = x[b]
        m["dt"] = delta_t[b]
        in_maps.append(m)

    kwargs = {}
    if TRACE:
        _install_trace_shims()
        kwargs["trace"] = True

    from concourse.bass_utils import run_bass_kernel_spmd
    res = run_bass_kernel_spmd(nc, in_maps, core_ids=list(range(B)), **kwargs)
    LAST_RESULTS = res

    out = np.stack([res.results[b]["out"] for b in range(B)]).astype(np.float32)
    fs = np.stack(
        [-res.results[b]["fs"].astype(np.float32).T.reshape(D) for b in range(B)]
    ).astype(np.float32)
    return out, fs


def _install_trace_shims():
    """Register the NTFF profile hook (missing antenv.axon_hooks shim) and
    skip the artifact upload. Only used when TRACE is enabled by test.py."""
    try:
        import antenv.axon_hooks  # noqa: F401
        return
    except ImportError:
        pass
    from trn_agent_boot.trn_boot import _ntff_profile_via_ctypes
    hook = _ntff_profile_via_ctypes("/opt/axon/libaxon_pjrt.so")
    mod = types.ModuleType("antenv.axon_hooks")
    mod.get_axon_ntff_profile_hook = lambda: hook
    sys.modules["antenv.axon_hooks"] = mod
    import concourse.bass_utils as bu
    bu.upload_artifacts = lambda tmpdir: "(upload skipped)"


# revision 15
# speedup vs baseline: 1.2993x; 1.1973x over previous
"""CTSelectiveSSMBlock Trainium2 kernel.

Strategy: data-parallel over batch (B=8 -> one batch element per NeuronCore).
Per core pipeline over t-tiles of 512 tokens:
  - DVE casts x / delta_t tiles to fp16 (delta_t fused with the max(dt,eps)
    clamp); PE transposes the fp16 tiles into [d, t] layout (1 cycle/row)
  - GEMM1/2 (fp16): update/gate pre-activations in [e, t] layout
  - ACT: lambda = exp(-decay*dt) straight from the transpose PSUM bank,
    sigmoid for the gate
  - DVE scalar_tensor_tensor: tmp = gate*(update+b_in); drv = (lam-1)*tmp
  - hardware tensor_tensor_scan for the recurrence (per 128-channel chunk,
    chained across tiles via the previous tile's last column)
  - GEMM3 (fp16): y = states @ -(I + W_out^T) using the negated-drive trick,
    bias b_out added via a K=1 ones-row matmul
  - LayerNorm via bn_stats/bn_aggr + ACT normalize, natural-layout DMA out
fp16 (not bf16): same PE/DMA cost, 4 extra mantissa bits; all tensors here
are O(1) so the reduced exponent range is irrelevant.
"""

import os
import sys
import types
import numpy as np
from contextlib import ExitStack

for _p in ("/opt/trn_rl_repo", "/opt/pypackages"):
    if _p not in sys.path and os.path.isdir(_p):
        sys.path.append(_p)

import concourse.bass as bass  # noqa: E402
import concourse.tile as tile  # noqa: E402
from concourse import bacc, mybir  # noqa: E402

F32 = mybir.dt.float32
F16 = mybir.dt.float16
AF = mybir.ActivationFunctionType
OP = mybir.AluOpType

B, T, D = 8, 4096, 1024
TT = 512           # tokens per tile
NA = TT // 128     # 128-token sub-chunks per tile
KC = D // 128      # channel chunks
EPS_DT = 1e-4
LN_EPS = 1e-5

# set by test harness for profiling; grading path leaves these off
TRACE = False
LAST_RESULTS = None

_BUILD_CACHE = {}


def _build(nt: int, apply_gb: bool):
    Tl = nt * TT
    nc = bacc.Bacc("TRN2", target_bir_lowering=False, debug=False)

    x_d = nc.dram_tensor("x", [Tl, D], F32, kind="ExternalInput")
    dt_d = nc.dram_tensor("dt", [Tl, D], F32, kind="ExternalInput")
    wig_d = nc.dram_tensor("w_ig", [D, 2 * D], F16, kind="ExternalInput")
    wom_d = nc.dram_tensor("w_om", [D, D], F16, kind="ExternalInput")
    big_d = nc.dram_tensor("b_ig", [128, 2 * KC], F32, kind="ExternalInput")
    nd_d = nc.dram_tensor("ndec", [128, KC], F32, kind="ExternalInput")
    bout_d = nc.dram_tensor("b_out", [1, D], F16, kind="ExternalInput")
    id_d = nc.dram_tensor("ident", [128, 128], F16, kind="ExternalInput")
    if apply_gb:
        gam_d = nc.dram_tensor("gam", [128, D], F32, kind="ExternalInput")
        bet_d = nc.dram_tensor("bet", [128, D], F32, kind="ExternalInput")
    out_d = nc.dram_tensor("out", [Tl, D], F32, kind="ExternalOutput")
    fs_d = nc.dram_tensor("fs", [128, KC], F16, kind="ExternalOutput")

    with ExitStack() as ctx:
        tc = ctx.enter_context(tile.TileContext(nc))
        wp = ctx.enter_context(tc.tile_pool(name="wp", bufs=1))
        xin = ctx.enter_context(tc.tile_pool(name="xin", bufs=4))
        xbp = ctx.enter_context(tc.tile_pool(name="xbp", bufs=2))
        xtp = ctx.enter_context(tc.tile_pool(name="xtp", bufs=2))
        stp = ctx.enter_context(tc.tile_pool(name="stp", bufs=2))
        lmp = ctx.enter_context(tc.tile_pool(name="lmp", bufs=10))
        ck3 = ctx.enter_context(tc.tile_pool(name="ck3", bufs=3))
        onp = ctx.enter_context(tc.tile_pool(name="onp", bufs=3))
        sm = ctx.enter_context(tc.tile_pool(name="sm", bufs=4))
        pt_ps = ctx.enter_context(tc.tile_pool(name="ptps", bufs=2, space="PSUM"))
        ug_ps = ctx.enter_context(tc.tile_pool(name="ugps", bufs=3, space="PSUM"))
        y_ps = ctx.enter_context(tc.tile_pool(name="yps", bufs=3, space="PSUM"))

        # ---- persistent weights / constants ----
        wig_s = wp.tile([128, KC * 2 * D], F16, tag="wig")
        nc.sync.dma_start(
            wig_s[:].rearrange("p (k e) -> p k e", k=KC),
            wig_d.rearrange("(k p) e -> p k e", p=128),
        )
        wom_s = wp.tile([128, KC * D], F16, tag="wom")
        nc.sync.dma_start(
            wom_s[:].rearrange("p (k e) -> p k e", k=KC),
            wom_d.rearrange("(k p) e -> p k e", p=128),
        )
        big_s = wp.tile([128, 2 * KC], F32, tag="big")
        nc.sync.dma_start(big_s[:], big_d[:])
        nd_s = wp.tile([128, KC], F32, tag="nd")
        nc.sync.dma_start(nd_s[:], nd_d[:])
        bout_s = wp.tile([1, D], F16, tag="bout")
        nc.sync.dma_start(bout_s[:], bout_d[:])
        id_s = wp.tile([128, 128], F16, tag="id")
        nc.sync.dma_start(id_s[:], id_d[:])
        ones_s = wp.tile([1, 128], F16, tag="ones")
        nc.vector.memset(ones_s[:], 1.0)
        if apply_gb:
            gam_s = wp.tile([128, D], F32, tag="gam")
            nc.sync.dma_start(gam_s[:], gam_d[:])
            bet_s = wp.tile([128, D], F32, tag="bet")
            nc.sync.dma_start(bet_s[:], bet_d[:])

        sT_prev = None
        for it in range(nt):
            t0 = it * TT
            # load + cast to fp16 (dt fused with the EPS clamp)
            x_b = xbp.tile([128, NA * D], F16, tag="x_b")
            dt_b = xbp.tile([128, NA * D], F16, tag="dt_b")
            for a in range(NA):
                x_nat = xin.tile([128, D], F32, tag="x_nat")
                nc.sync.dma_start(x_nat[:], x_d[t0 + a * 128: t0 + (a + 1) * 128, :])
                nc.vector.tensor_copy(x_b[:, a * D:(a + 1) * D], x_nat[:])
                dt_nat = xin.tile([128, D], F32, tag="dt_nat")
                nc.sync.dma_start(dt_nat[:], dt_d[t0 + a * 128: t0 + (a + 1) * 128, :])
                nc.vector.tensor_scalar_max(dt_b[:, a * D:(a + 1) * D], dt_nat[:], EPS_DT)

            xt = xtp.tile([128, KC * TT], F16, tag="xt")
            lams = []
            for k in range(KC):
                # x^T chunk: transpose NA fp16 blocks into one PSUM tile
                ptx = pt_ps.tile([128, TT], F16, tag="pt")
                for a in range(NA):
                    nc.tensor.transpose(
                        ptx[:, a * 128:(a + 1) * 128],
                        x_b[:, a * D + k * 128: a * D + (k + 1) * 128],
                        id_s[:],
                    )
                nc.vector.tensor_copy(xt[:, k * TT:(k + 1) * TT], ptx[:])
                # dt^T chunk: lambda computed by ACT straight from PSUM
                ptd = pt_ps.tile([128, TT], F16, tag="pt")
                for a in range(NA):
                    nc.tensor.transpose(
                        ptd[:, a * 128:(a + 1) * 128],
                        dt_b[:, a * D + k * 128: a * D + (k + 1) * 128],
                        id_s[:],
                    )
                lam = lmp.tile([128, TT], F16, tag="lam")
                nc.scalar.activation(lam[:], ptd[:], AF.Exp, scale=nd_s[:, k:k + 1])
                lams.append(lam)

            sT = stp.tile([128, KC * TT], F16, tag="sT")
            for c in range(KC):
                # update pre-activation (chunk c of output channels)
                psu = ug_ps.tile([128, TT], F32, tag="ug")
                for k in range(KC):
                    nc.tensor.matmul(
                        psu[:],
                        wig_s[:, k * 2 * D + c * 128: k * 2 * D + (c + 1) * 128],
                        xt[:, k * TT:(k + 1) * TT],
                        start=(k == 0), stop=(k == KC - 1),
                    )
                # gate pre-activation
                psg = ug_ps.tile([128, TT], F32, tag="ug")
                for k in range(KC):
                    nc.tensor.matmul(
                        psg[:],
                        wig_s[:, k * 2 * D + D + c * 128: k * 2 * D + D + (c + 1) * 128],
                        xt[:, k * TT:(k + 1) * TT],
                        start=(k == 0), stop=(k == KC - 1),
                    )
                gat = ck3.tile([128, TT], F16, tag="gat")
                nc.scalar.activation(gat[:], psg[:], AF.Sigmoid,
                                     bias=big_s[:, KC + c:KC + c + 1])
                # tmp = gate * (update_pre + b_in)   (reads PSUM directly)
                tmp = ck3.tile([128, TT], F16, tag="tmp")
                nc.vector.scalar_tensor_tensor(
                    tmp[:], psu[:], big_s[:, c:c + 1], gat[:], op0=OP.add, op1=OP.mult
                )
                # drv = (lam - 1) * tmp  == -drive
                drv = ck3.tile([128, TT], F16, tag="drv")
                nc.vector.scalar_tensor_tensor(
                    drv[:], lams[c][:], 1.0, tmp[:], op0=OP.subtract, op1=OP.mult
                )
                init = 0.0 if it == 0 else sT_prev[:, c * TT + TT - 1: c * TT + TT]
                nc.vector.tensor_tensor_scan(
                    sT[:, c * TT:(c + 1) * TT], lams[c][:], drv[:], init,
                    op0=OP.mult, op1=OP.add,
                )

            for a in range(NA):
                on = onp.tile([128, D], F32, tag="on")
                yps = []
                for h in range(2):
                    yp = y_ps.tile([128, 512], F32, tag="y")
                    for k in range(KC):
                        nc.tensor.matmul(
                            yp[:],
                            sT[:, k * TT + a * 128: k * TT + (a + 1) * 128],
                            wom_s[:, k * D + h * 512: k * D + (h + 1) * 512],
                            start=(k == 0), stop=False,
                        )
                    nc.tensor.matmul(
                        yp[:], ones_s[:], bout_s[:, h * 512:(h + 1) * 512],
                        start=False, stop=True,
                    )
                    yps.append(yp)
                stats = sm.tile([128, 12], F32, tag="stats")
                nc.vector.bn_stats(stats[:, 0:6], yps[0][:])
                nc.vector.bn_stats(stats[:, 6:12], yps[1][:])
                mv = sm.tile([128, 2], F32, tag="mv")
                nc.vector.bn_aggr(mv[:], stats[:])
                ve = sm.tile([128, 1], F32, tag="ve")
                nc.vector.tensor_scalar_add(ve[:], mv[:, 1:2], LN_EPS)
                sd = sm.tile([128, 1], F32, tag="sd")
                nc.scalar.sqrt(sd[:], ve[:])
                rs = sm.tile([128, 1], F32, tag="rs")
                nc.vector.reciprocal(rs[:], sd[:])
                nm = sm.tile([128, 1], F32, tag="nm")
                nc.vector.scalar_tensor_tensor(
                    nm[:], mv[:, 0:1], -1.0, rs[:], op0=OP.mult, op1=OP.mult
                )
                for h in range(2):
                    nc.vector.tensor_scalar(
                        on[:, h * 512:(h + 1) * 512], yps[h][:], rs[:], nm[:],
                        op0=OP.mult, op1=OP.add,
                    )
                if apply_gb:
                    nc.vector.tensor_mul(on[:], on[:], gam_s[:])
                    nc.vector.tensor_add(on[:], on[:], bet_s[:])
                nc.sync.dma_start(out_d[t0 + a * 128: t0 + (a + 1) * 128, :], on[:])

            sT_prev = sT

        nc.sync.dma_start(
            fs_d.rearrange("p (k o) -> p k o", o=1),
            sT_prev[:].rearrange("p (k t) -> p k t", k=KC)[:, :, TT - 1:TT],
        )

    nc.compile()
    return nc


def get_nc(nt: int = T // TT, apply_gb: bool = False):
    key = (nt, apply_gb)
    if key not in _BUILD_CACHE:
        _BUILD_CACHE[key] = _build(nt, apply_gb)
    return _BUILD_CACHE[key]


def host_prep(log_decay, W_in, b_in, W_gate, b_gate, W_out, b_out, gamma, beta):
    f32 = np.float32
    f16 = np.float16
    decay = np.logaddexp(log_decay.astype(f32), f32(0.0)).astype(f32) + f32(1e-4)
    ndec = np.ascontiguousarray((-decay).reshape(KC, 128).T)
    w_ig = np.ascontiguousarray(
        np.concatenate([np.asarray(W_in, f32).T, np.asarray(W_gate, f32).T], axis=1)
    ).astype(f16)
    w_om = np.ascontiguousarray(
        -(np.eye(D, dtype=f32) + np.asarray(W_out, f32).T)
    ).astype(f16)
    b_ig = np.ascontiguousarray(
        np.concatenate(
            [np.asarray(b_in, f32).reshape(KC, 128).T,
             np.asarray(b_gate, f32).reshape(KC, 128).T], axis=1)
    )
    b_out_row = np.ascontiguousarray(np.asarray(b_out, f32).reshape(1, D)).astype(f16)
    ident = np.eye(128, dtype=f16)
    gamma = np.asarray(gamma, f32)
    beta = np.asarray(beta, f32)
    apply_gb = not (np.all(gamma == 1.0) and np.all(beta == 0.0))
    common = {
        "w_ig": w_ig, "w_om": w_om, "b_ig": b_ig, "ndec": ndec,
        "b_out": b_out_row, "ident": ident,
    }
    if apply_gb:
        common["gam"] = np.ascontiguousarray(np.broadcast_to(gamma, (128, D)))
        common["bet"] = np.ascontiguousarray(np.broadcast_to(beta, (128, D)))
    return common, apply_gb


def kernel(x, delta_t, log_decay, W_in, b_in, W_gate, b_gate, W_out, b_out,
           gamma, beta):
    global LAST_RESULTS
    x = np.ascontiguousarray(np.asarray(x, np.float32))
    delta_t = np.ascontiguousarray(np.asarray(delta_t, np.float32))
    common, apply_gb = host_prep(log_decay, W_in, b_in, W_gate, b_gate,
                                 W_out, b_out, gamma, beta)
    nc = get_nc(T // TT, apply_gb)

    in_maps = []
    for b in range(B):
        m = dict(common)
        m["x"] = x[b]
        m["dt"] = delta_t[b]
        in_maps.append(m)

    kwargs = {}
    if TRACE:
        _install_trace_shims()
        kwargs["trace"] = True

    from concourse.bass_utils import run_bass_kernel_spmd
    res = run_bass_kernel_spmd(nc, in_maps, core_ids=list(range(B)), **kwargs)
    LAST_RESULTS = res

    out = np.stack([res.results[b]["out"] for b in range(B)]).astype(np.float32)
    fs = np.stack(
        [-res.results[b]["fs"].astype(np.float32).T.reshape(D) for b in range(B)]
    ).astype(np.float32)
    return out, fs


def _install_trace_shims():
    """Register the NTFF profile hook (missing antenv.axon_hooks shim) and
    skip the artifact upload. Only used when TRACE is enabled by test.py."""
    try:
        import antenv.axon_hooks  # noqa: F401
        return
    except ImportError:
        pass
    from trn_agent_boot.trn_boot import _ntff_profile_via_ctypes
    hook = _ntff_profile_via_ctypes("/opt/axon/libaxon_pjrt.so")
    mod = types.ModuleType("antenv.axon_hooks")
    mod.get_axon_ntff_profile_hook = lambda: hook
    sys.modules["antenv.axon_hooks"] = mod
    import concourse.bass_utils as bu
    bu.upload_artifacts = lambda tmpdir: "(upload skipped)"
